# revision 4
# baseline (speedup 1.0000x reference)
"""Dice metric kernel v9 for Trainium2 (Bass/Tile), 8-core data parallel.

Encoding trick: host packs quantized logits + class index into int16:
    q_c = 8*clip(round(512*x_c), -2047, 2046) + 16384 + c
q is positive, strictly monotone in quantized x, and argmax tie-break picks
the highest class index. A single max-tree then yields both the max value
and the argmax:  pred = max_c(q_c) & 7.

Per chunk (1024 cols) the chip computes:
  li  = maxq & 7                      (pred label plane, int16)
  ji  = li + 8*tgt                    (joint label in 0..63)
  tps_c mask = [ji == 9c]  c=1..7    4x-mode TS is_equal -> fp16 planes,
  pm_7 mask  = [li == 7]             summed by PE one-hot matmuls into a
                                      per-sample PSUM[8,512], harvested by
                                      one ACT Copy+accum -> [8,1]
  m_c = sum relu(li - c + 0.5)  c=1..6  hinge moments on ACT (per chunk)
Host: pmc_1..6 from the triangular hinge system (pmc_7 direct), tmc from a
bincount of targets, dice = mean_c 2*tps/(pmc+tmc+eps).

Sharding: batch 16 -> 2 samples per core on 8 cores; host concatenates.
"""

import numpy as np

import concourse.bacc as bacc
import concourse.mybir as mybir
import concourse.tile as tile
from concourse.bass_utils import run_bass_kernel_spmd

B, C, H, W = 16, 8, 512, 512
NCORES = 8
BPC = B // NCORES          # samples per core
P = 128                    # SBUF partitions
F = (H * W) // P           # free dim per plane (2048)
WCH = 1024                 # chunk width
NCH = F // WCH             # chunks per sample
EPS = 1e-5
K_ACTS = {0: 6, 1: 2}      # per-sample: pmc classes 1..K via ACT hinges, rest via DVE masks

_f32 = mybir.dt.float32
_f16 = mybir.dt.float16
_i16 = mybir.dt.int16
_alu = mybir.AluOpType
_act = mybir.ActivationFunctionType


def _build_nc():
    nc = bacc.Bacc(None, target_bir_lowering=False, debug=False)
    q_dram = nc.dram_tensor("q", [BPC, C, P, F], _i16, kind="ExternalInput")
    t8_dram = nc.dram_tensor("t8", [BPC, P, F], _i16, kind="ExternalInput")
    oh_dram = nc.dram_tensor("oh", [P, BPC * NCH * 8], _f32, kind="ExternalOutput")
    op_dram = nc.dram_tensor("op", [12, BPC], _f32, kind="ExternalOutput")

    with tile.TileContext(nc) as tc:
        with (
            tc.tile_pool(name="qp", bufs=3) as qp,
            tc.tile_pool(name="mk", bufs=3) as mk,
            tc.tile_pool(name="wk", bufs=2) as wk,
            tc.tile_pool(name="cst", bufs=1) as cst,
            tc.tile_pool(name="ps", bufs=2, space="PSUM") as ps,
        ):
            ecs = cst.tile([P, 12, 12], _f16)
            nc.gpsimd.memset(ecs[:], 0.0)
            for ci in range(12):
                nc.gpsimd.memset(ecs[:, ci, ci:ci + 1], 1.0)
            biases = cst.tile([P, 8], _f32)
            for c in range(1, 8):
                nc.gpsimd.memset(biases[:, c:c + 1], -(float(c) - 0.5))
            acch = cst.tile([P, BPC * NCH * 8], _f32)
            accp = cst.tile([12, BPC], _f32)
            hjunk = cst.tile([P, 1536], _f16)
            pjunk = cst.tile([12, 512], _f32)
            pjunk2 = cst.tile([12, 512], _f32)

            # interleaved schedule: big chunks first, samples alternating
            SCHED = [(0, 0, 0, 1024), (1, 0, 0, 1024), (0, 1, 1024, 1024), (1, 1, 1024, 1024)]
            pps = {}
            nmms = {}
            tbs = {}
            # mask rows per sample: row 0 = pm7, rows 1..7 = tps, row 8 = pm6, row 9 = pm5
            ROWS = {0: list(range(1, 8)) + [0], 1: list(range(1, 8)) + [0, 8, 9, 10, 11]}
            nmm_lasts = {b: (F // 512) * len(ROWS[b]) - 1 for b in range(BPC)}
            for (b, k, cs, W_) in SCHED:
                if True:
                    if b not in pps:
                        pp_new = ps.tile([12, 512], _f32, tag=f"pp{b}")
                        pps[b] = pp_new
                        nmms[b] = [0]
                    pp = pps[b]
                    nmm = nmms[b]
                    hs = slice(cs, cs + W_)
                    xb = qp.tile([P, C, W_], _i16, tag="xb")
                    # two batched DMA issues per chunk (planes 0-3, 4-7)
                    nc.sync.dma_start(xb[:, 0:4, :], q_dram[b, 0:4, :, hs].rearrange("c p w -> p c w"))
                    nc.sync.dma_start(xb[:, 4:8, :], q_dram[b, 4:8, :, hs].rearrange("c p w -> p c w"))
                    if b not in tbs:
                        tb_new = qp.tile([P, F], _i16, tag=f"tb{b}")
                        nc.sync.dma_start(tb_new[:], t8_dram[b][:, :])
                        tbs[b] = tb_new
                    tb = tbs[b]

                    t1 = wk.tile([P, 4, W_], _i16, tag="t1")
                    nc.vector.tensor_tensor(out=t1[:, 0:2, :], in0=xb[:, 0:2, :],
                                            in1=xb[:, 2:4, :], op=_alu.max)
                    nc.vector.tensor_tensor(out=t1[:, 2:4, :], in0=xb[:, 4:6, :],
                                            in1=xb[:, 6:8, :], op=_alu.max)
                    t2 = wk.tile([P, 2, W_], _i16, tag="t2")
                    nc.vector.tensor_tensor(out=t2[:], in0=t1[:, 0:2, :],
                                            in1=t1[:, 2:4, :], op=_alu.max)
                    mxq = wk.tile([P, W_], _i16, tag="mx")
                    nc.vector.tensor_tensor(out=mxq[:], in0=t2[:, 0, :],
                                            in1=t2[:, 1, :], op=_alu.max)
                    li = wk.tile([P, W_], _i16, tag="li")
                    nc.vector.tensor_scalar(out=li[:], in0=mxq[:], scalar1=7,
                                            scalar2=None, op0=_alu.bitwise_and)
                    jb = wk.tile([P, W_], _i16, tag="jb")
                    nc.vector.tensor_tensor(out=jb[:], in0=li[:], in1=tb[:, hs], op=_alu.add)

                    msk = mk.tile([P, 12, W_], _f16, tag="msk")
                    for c in range(1, 8):
                        nc.vector.tensor_scalar(out=msk[:, c, :], in0=jb[:],
                                                scalar1=9 * c, scalar2=None,
                                                op0=_alu.is_equal)
                    nc.vector.tensor_scalar(out=msk[:, 0, :], in0=li[:],
                                            scalar1=7, scalar2=None,
                                            op0=_alu.is_equal)
                    if b == 1:
                        for r, cv in ((8, 6), (9, 5), (10, 4), (11, 3)):
                            nc.vector.tensor_scalar(out=msk[:, r, :], in0=li[:],
                                                    scalar1=cv, scalar2=None,
                                                    op0=_alu.is_equal)

                    # PE: one-hot stationary routes each mask's colsum to its row
                    for c in ROWS[b]:
                        for j in range(W_ // 512):
                            nc.tensor.matmul(pp[:], ecs[:, c, :],
                                             msk[:, c, 512 * j:512 * (j + 1)],
                                             start=(nmm[0] == 0),
                                             stop=(nmm[0] == nmm_lasts[b]))
                            nmm[0] += 1

                    # ACT hinge moments on this chunk's li
                    base = (b * NCH + k) * 8
                    for c in range(1, K_ACTS[b] + 1):
                        nc.scalar.activation(hjunk[:, 0:W_], li[:], _act.Relu,
                                             bias=biases[:, c:c + 1], scale=1.0,
                                             accum_out=acch[:, base + c:base + c + 1])

                    # after the last chunk of sample b: harvest + partial output
                    if k == NCH - 1:
                        if b == 0:
                            nc.scalar.activation(pjunk[:], pp[:], _act.Copy,
                                                 accum_out=accp[:, b:b + 1])
                        else:
                            # DVE is idle at the end; ACT is the tail
                            nc.vector.tensor_scalar(out=pjunk2[:], in0=pp[:],
                                                    scalar1=1.0, scalar2=0.0,
                                                    op0=_alu.mult, op1=_alu.add,
                                                    accum_out=accp[:, b:b + 1])
                        lo, hi = b * NCH * 8, (b + 1) * NCH * 8
                        nc.sync.dma_start(oh_dram[:, lo:hi], acch[:, lo:hi])
                        nc.sync.dma_start(op_dram[:, b:b + 1], accp[:, b:b + 1])

    nc.compile()
    return nc


_NC_CACHE = {}


def _get_nc():
    if "nc" not in _NC_CACHE:
        _NC_CACHE["nc"] = _build_nc()
    return _NC_CACHE["nc"]


def make_in_maps(inputs: np.ndarray, targets: np.ndarray) -> list:
    x = np.asarray(inputs, dtype=np.float32)
    qv = np.clip(np.rint(x * 512.0), -2047.0, 2046.0).astype(np.int16)
    q = (qv * 8 + np.int16(16384)
         + np.arange(8, dtype=np.int16)[None, :, None, None])
    q = np.ascontiguousarray(q).reshape(NCORES, BPC, C, P, F)
    t = np.ascontiguousarray(targets).reshape(B, P, F).astype(np.int16)
    t8 = (t * 8).reshape(NCORES, BPC, P, F)
    return [{"q": q[i], "t8": t8[i]} for i in range(NCORES)]


def _postprocess(res, targets) -> np.ndarray:
    t = np.asarray(targets).reshape(B, H * W)
    out = np.zeros(B, dtype=np.float64)
    for i in range(NCORES):
        oh = res.results[i]["oh"]   # [P, BPC*NCH*8]
        op = res.results[i]["op"]   # [8, BPC]
        for b in range(BPC):
            s = i * BPC + b
            tps = op[1:8, b].astype(np.float64)
            K_ACT = K_ACTS[b]
            h = np.zeros(7)
            h[6] = float(op[0, b])  # pmc_7 direct
            if b == 1:
                for r, cv in ((8, 6), (9, 5), (10, 4), (11, 3)):
                    h[cv - 1] = float(op[r, b])
            # hinge moments summed over chunks and partitions
            m = np.zeros(8)
            for k in range(NCH):
                base = (b * NCH + k) * 8
                m += oh[:, base:base + 8].sum(axis=0).astype(np.float64)
            # m_c = sum_{j>=c} (j-c+0.5) h_j  for c=1..K_ACT; higher h known
            for c in range(K_ACT, 0, -1):
                acc = m[c]
                for j in range(c + 1, 8):
                    acc -= (j - c + 0.5) * h[j - 1]
                h[c - 1] = acc / 0.5
            tmc = np.bincount(t[s], minlength=8)[1:8].astype(np.float64)
            dice = 2.0 * tps / (h + tmc + EPS)
            out[s] = dice.mean()
    return out.astype(np.float32)


def kernel(inputs: np.ndarray, targets: np.ndarray) -> np.ndarray:
    in_maps = make_in_maps(inputs, targets)
    nc = _get_nc()
    res = run_bass_kernel_spmd(nc, in_maps, list(range(NCORES)))
    return _postprocess(res, targets)
